# revision 1
# baseline (speedup 1.0000x reference)
"""Trainium2 Bass kernel for nn_NoFoDifformer_FourierKAN (8-core SPMD).

Sharding: u and nodes row-wise across 8 cores (1250 rows each). The [d,d]
K^T V Gram matrices and the chunked u^T h partial sums are all-reduced;
small weights are replicated; per-core outputs are produced TRANSPOSED
([d, n_loc]) and transposed+concatenated on the host.

Key structure (v2):
- u is read from HBM exactly once per core via a single SWDGE cast-DMA per
  chunk (fp32 -> bf16 into SBUF); pass-1 matmuls consume the bf16 tiles
  directly, then the same tiles are written (bf16->bf16, HWDGE) to a DRAM
  staging buffer that pass-2 reads back with tall xbar-transposed loads.
- The epilogue (attention apply, residuals, LayerNorms, FFN) runs entirely
  in transposed activation layout [d, i] with weight-stationary matmuls:
  no per-row-tile PE transposes; LN stats via ones-matmul partition sums.
- LayerNorm affine params are folded into downstream projection weights.
"""

import math
from contextlib import ExitStack

import numpy as np

N_FULL = 10000
NF_FULL = 512
D = 128
CORES_FULL = 8
CHUNK_FULL = 2048
LAMBDA_INIT = 0.2


def _ceil_div(a, b):
    return (a + b - 1) // b


def _splits(total, step):
    return [(o, min(step, total - o)) for o in range(0, total, step)]


def build_kernel(N=N_FULL, NF=NF_FULL, CORES=CORES_FULL, CHUNK=CHUNK_FULL,
                 debug=False):
    import concourse.bacc as bacc
    import concourse.tile as tile
    from concourse import mybir
    from concourse.masks import make_identity

    dt = mybir.dt
    f32 = dt.float32
    f32r = dt.float32r
    bf16 = dt.bfloat16
    AF = mybir.ActivationFunctionType
    ALU = mybir.AluOpType
    AX = mybir.AxisListType

    NLOC = N // CORES                  # 1250
    ROWS = _splits(NLOC, 128)          # row tiles per core (9x128 + 98)
    NT = len(ROWS)
    KX = NF // 128                     # x feature k-tiles
    assert KX * 128 == NF
    CHUNKS = _splits(N, CHUNK)         # j chunks
    NCH = len(CHUNKS)
    NSUB = _ceil_div(N, 128)           # global 128-wide j subtiles
    N_PAD = NSUB * 128
    NSUB_C = _ceil_div(CHUNK, 128)     # max j subtiles per chunk
    NLOC_PAD = _ceil_div(NLOC, 16) * 16  # xbar tall-read row pad (1264)
    TG_FULL = N // 128
    TG_REM = N - TG_FULL * 128
    IBLK = _splits(NLOC, 512)          # pass-2 output i blocks
    BLK = _splits(NLOC, 512)           # [128, NLOC] op blocks
    DEPTH = 2                          # pass2 runs DEPTH chunks behind pass1
    assert TG_FULL <= 128
    rg = [list(range(CORES))]
    shared_space = "Shared" if CORES > 4 else "Local"

    nc = bacc.Bacc("TRN2", target_bir_lowering=False, debug=False,
                   num_devices=CORES)

    # ---------------- DRAM I/O ----------------
    def din(name, shape):
        return nc.dram_tensor(name, list(shape), f32, kind="ExternalInput")

    t_x = din("x", (NLOC, NF))
    t_u = nc.dram_tensor("u", [NLOC, N], bf16, kind="ExternalInput")
    t_uT = nc.dram_tensor("uT", [N, NLOC], bf16, kind="ExternalInput")
    t_e = din("e", (N,))
    t_few1 = din("fe_w1", (NF, D)); t_feb1 = din("fe_b1", (D,))
    t_few2 = din("fe_w2", (D, D)); t_feb2 = din("fe_b2", (D,))
    t_kana = din("kan_a", (10,)); t_kanb = din("kan_b", (10,))
    t_kanbias = din("kan_bias", (1,)); t_alpha = din("alpha_w", (1, 1))
    t_mg = din("mha_ln_g", (D,)); t_mb = din("mha_ln_b", (D,))
    t_fg = din("ffn_ln_g", (D,)); t_fb = din("ffn_ln_b", (D,))
    t_q1w = din("q1_w", (D, D)); t_q1b = din("q1_b", (D,))
    t_k1w = din("k1_w", (D, D)); t_k1b = din("k1_b", (D,))
    t_q2w = din("q2_w", (D, D)); t_q2b = din("q2_b", (D,))
    t_k2w = din("k2_w", (D, D)); t_k2b = din("k2_b", (D,))
    t_vw = din("v_w", (D, D)); t_vb = din("v_b", (D,))
    t_ag = din("attn_ln_g", (D,)); t_ab = din("attn_ln_b", (D,))
    t_ow = din("out_w", (D, D)); t_ob = din("out_b", (D,))
    t_lq1 = din("lq1", (D,)); t_lk1 = din("lk1", (D,))
    t_lq2 = din("lq2", (D,)); t_lk2 = din("lk2", (D,))
    t_f1w = din("ffn1_w", (D, D)); t_f1b = din("ffn1_b", (D,))
    t_f2w = din("ffn2_w", (D, D)); t_f2b = din("ffn2_b", (D,))
    t_out = nc.dram_tensor("out", [D, NLOC], f32, kind="ExternalOutput")
    if debug:
        bf16_ = __import__("concourse.mybir", fromlist=["dt"]).dt.bfloat16
        t_dhT = nc.dram_tensor("d_hT", [D, NLOC], f32, kind="ExternalOutput")
        t_dhnT = nc.dram_tensor("d_hnT", [D, NLOC], bf16_,
                                kind="ExternalOutput")
        t_dp1 = nc.dram_tensor("d_p1", [D, CHUNK], bf16_,
                               kind="ExternalOutput")
        t_dz = nc.dram_tensor("d_z", [D, _ceil_div(CHUNK, 128) * D], bf16_,
                              kind="ExternalOutput")
        t_duT = nc.dram_tensor("d_uT", [D, _ceil_div(N // CORES, 16) * 16],
                               bf16_, kind="ExternalOutput")
        t_dhaT = nc.dram_tensor("d_haT", [D, NLOC], f32,
                                kind="ExternalOutput")
        t_dhenc = nc.dram_tensor("d_henc", [D, NLOC], f32,
                                 kind="ExternalOutput")
        t_dfT = nc.dram_tensor("d_fT", [D, NLOC], bf16_,
                               kind="ExternalOutput")
        NCH_ = len(_splits(N, CHUNK))
        NLP_ = _ceil_div(N // CORES, 16) * 16
        t_duTall = nc.dram_tensor("d_uTall", [D, NCH_ * NLP_], bf16_,
                                  kind="ExternalOutput")
        t_dzall = nc.dram_tensor("d_zall", [D, NCH_ * D], bf16_,
                                 kind="ExternalOutput")

    with tile.TileContext(nc) as tc, ExitStack() as ctx:
        wpool = ctx.enter_context(tc.tile_pool(name="wpool", bufs=1))
        rowtmp = ctx.enter_context(tc.tile_pool(name="rowtmp", bufs=3))
        ustream = ctx.enter_context(tc.tile_pool(name="ustream", bufs=12))
        uTp = ctx.enter_context(tc.tile_pool(name="uTp", bufs=6))
        zcp = ctx.enter_context(tc.tile_pool(name="zcp", bufs=1))
        z16p = ctx.enter_context(tc.tile_pool(name="z16p", bufs=2))
        p1sbp = ctx.enter_context(tc.tile_pool(name="p1sbp", bufs=2))
        dram = ctx.enter_context(tc.tile_pool(name="dram", bufs=1, space="DRAM"))
        ps_p1 = ctx.enter_context(tc.tile_pool(name="ps_p1", bufs=1, space="PSUM"))
        ps_p2 = ctx.enter_context(tc.tile_pool(name="ps_p2", bufs=3, space="PSUM"))
        ps_mm = ctx.enter_context(tc.tile_pool(name="ps_mm", bufs=2, space="PSUM"))
        ps_t = ctx.enter_context(tc.tile_pool(name="ps_t", bufs=2, space="PSUM"))

        def p1_tile(w):
            return ps_p1.tile([128, 512], f32, tag="p1",
                              name=f"p1_{nc.next_id()}")[:, :w]

        def p2_tile(w):
            return ps_p2.tile([128, 512], f32, tag="p2",
                              name=f"p2_{nc.next_id()}")[:, :w]

        def mm_tile(p, w):
            return ps_mm.tile([128, 512], f32, tag="mmp",
                              name=f"mm_{nc.next_id()}")[:p, :w]

        def tb_tile(p, w):
            return ps_t.tile([128, 128], bf16, tag="pstb",
                             name=f"pstb_{nc.next_id()}")[:p, :w]

        def wtile(shape, dtype, name):
            return wpool.tile(shape, dtype, tag=name, name=name)

        def rtile(shape, dtype, tag):
            return rowtmp.tile(shape, dtype, tag=tag,
                               name=f"{tag}_{nc.next_id()}")

        def T(out_psum, in_sbuf, identity):
            nc.tensor.matmul(out_psum, in_sbuf, identity, is_transpose=True)

        # ================= constants & weights =================
        ident = wtile([128, 128], f32, "ident")
        make_identity(nc, ident[:])
        identb = wtile([128, 128], bf16, "identb")
        make_identity(nc, identb[:])

        ones_row = wtile([1, 128], f32, "ones_row")
        nc.vector.memset(ones_row[:], 1.0)
        ones_row_b = wtile([1, 128], bf16, "ones_row_b")
        nc.vector.memset(ones_row_b[:], 1.0)
        oinv_col_b = wtile([128, 1], bf16, "oinv_col_b")
        nc.vector.memset(oinv_col_b[:], 1.0 / 128.0)
        eps_col = wtile([128, 1], f32, "eps_col")
        nc.vector.memset(eps_col[:], 1e-5)
        c08_col = wtile([128, 1], f32, "c08_col")
        nc.vector.memset(c08_col[:], 1.0 - LAMBDA_INIT)
        one_col = wtile([128, 1], f32, "one_col")
        nc.vector.memset(one_col[:], 1.0)
        laminit_c = wtile([1, 1], f32, "laminit_c")
        nc.vector.memset(laminit_c[:], LAMBDA_INIT)

        def ldw(name, dram_t, shape, rearr=None, **kw):
            t = wtile(shape, f32, name)
            src = dram_t[:] if rearr is None else dram_t[:].rearrange(rearr, **kw)
            nc.scalar.dma_start(out=t[:], in_=src)
            return t

        few1b = wtile([128, KX, D], bf16, "few1b")
        for kt in range(KX):
            nc.gpsimd.dma_start(out=few1b[:, kt, :],
                                in_=t_few1[kt * 128:(kt + 1) * 128, :])
        few2b = wtile([128, D], bf16, "few2b")
        nc.gpsimd.dma_start(out=few2b[:], in_=t_few2[:])
        f2wb = wtile([128, D], bf16, "f2wb")
        nc.gpsimd.dma_start(out=f2wb[:], in_=t_f2w[:])
        q1w = ldw("q1w", t_q1w, [128, D])
        k1w = ldw("k1w", t_k1w, [128, D])
        q2w = ldw("q2w", t_q2w, [128, D])
        k2w = ldw("k2w", t_k2w, [128, D])
        vw = ldw("vw", t_vw, [128, D])
        ow = ldw("ow", t_ow, [128, D])
        f1w = ldw("f1w", t_f1w, [128, D])

        def ldcol(name, dram_t):
            t = wtile([128, 1], f32, name)
            nc.scalar.dma_start(out=t[:],
                                in_=dram_t[:].rearrange("(p x) -> p x", x=1))
            return t

        feb1_c = ldcol("feb1_c", t_feb1)
        feb2_c = ldcol("feb2_c", t_feb2)
        mg_c = ldcol("mg_c", t_mg); mb_c = ldcol("mb_c", t_mb)
        fg_c = ldcol("fg_c", t_fg); fb_c = ldcol("fb_c", t_fb)
        ag_c = ldcol("ag_c", t_ag); ab_c = ldcol("ab_c", t_ab)
        q1b_c = ldcol("q1b_c", t_q1b); q2b_c = ldcol("q2b_c", t_q2b)
        ob_c = ldcol("ob_c", t_ob)
        f1b_c = ldcol("f1b_c", t_f1b)
        f2b_c = ldcol("f2b_c", t_f2b)

        def ldrow(name, dram_t, w=128):
            t = wtile([1, w], f32, name)
            nc.scalar.dma_start(out=t[:],
                                in_=dram_t[:].rearrange("(x p) -> x p", x=1))
            return t

        k1b_r = ldrow("k1b_r", t_k1b); k2b_r = ldrow("k2b_r", t_k2b)
        vb_r = ldrow("vb_r", t_vb)
        lq1_r = ldrow("lq1_r", t_lq1); lk1_r = ldrow("lk1_r", t_lk1)
        lq2_r = ldrow("lq2_r", t_lq2); lk2_r = ldrow("lk2_r", t_lk2)
        kana_r = ldrow("kana_r", t_kana, 10)
        kanb_r = ldrow("kanb_r", t_kanb, 10)
        kbias_r = ldrow("kbias_r", t_kanbias, 1)
        alpha_r = wtile([1, 1], f32, "alpha_r")
        nc.scalar.dma_start(out=alpha_r[:], in_=t_alpha[:])

        def ldbcast(name, dram_t):
            t = wtile([128, D], f32, name)
            nc.scalar.dma_start(out=t[:], in_=dram_t[:].partition_broadcast(128))
            return t

        feb2_B = ldbcast("feb2_B", t_feb2)

        # ---------- scalars: lambda ----------
        srow = wtile([1, 8], f32, "srow")
        nc.vector.memset(srow[:], 0.0)
        tmpr = wtile([1, 128], f32, "tmpr")
        lam1 = wtile([1, 1], f32, "lam1")
        lam2 = wtile([1, 1], f32, "lam2")
        nc.vector.tensor_mul(tmpr[:], lq1_r[:], lk1_r[:])
        nc.vector.tensor_reduce(lam1[:], tmpr[:], axis=AX.X, op=ALU.add)
        nc.scalar.activation(lam1[:], lam1[:], AF.Exp)
        nc.vector.tensor_mul(tmpr[:], lq2_r[:], lk2_r[:])
        nc.vector.tensor_reduce(lam2[:], tmpr[:], axis=AX.X, op=ALU.add)
        nc.scalar.activation(lam2[:], lam2[:], AF.Exp)
        nc.vector.tensor_sub(srow[:, 0:1], lam1[:], lam2[:])
        nc.vector.tensor_add(srow[:, 0:1], srow[:, 0:1], laminit_c[:])  # lam_full
        nc.scalar.mul(srow[:, 1:2], srow[:, 0:1], -1.0)            # -lam_full
        nc.vector.tensor_copy(srow[:, 2:3], alpha_r[:])
        nc.vector.tensor_copy(srow[:, 3:4], kbias_r[:])

        ps_b = mm_tile(128, 28)
        nc.tensor.matmul(ps_b[:, 0:8], ones_row[:], srow[:],
                         start=True, stop=False)
        nc.tensor.matmul(ps_b[:, 8:18], ones_row[:], kana_r[:],
                         start=False, stop=False)
        nc.tensor.matmul(ps_b[:, 18:28], ones_row[:], kanb_r[:],
                         start=False, stop=True)
        sB = wtile([128, 28], f32, "sB")
        nc.vector.tensor_copy(sB[:], ps_b)
        neglam_c = sB[:, 1:2]
        alpha_c = sB[:, 2:3]
        kbias_c = sB[:, 3:4]

        # ---------- new_e from e (FourierKAN), layout [128, NSUB] ----------
        eT = wtile([128, NSUB], f32, "eT")
        nc.vector.memset(eT[:], 0.0)
        eload = wtile([max(TG_FULL, 1), 128], f32, "eload")
        nc.scalar.dma_start(
            out=eload[:TG_FULL],
            in_=t_e[: TG_FULL * 128].rearrange("(t p) -> t p", p=128))
        pse = mm_tile(128, TG_FULL)
        T(pse, eload[:TG_FULL], ident[:TG_FULL, :TG_FULL])
        nc.vector.tensor_copy(eT[:, :TG_FULL], pse)
        if TG_REM > 0:
            erem = wtile([1, TG_REM], f32, "erem")
            nc.scalar.dma_start(
                out=erem[:],
                in_=t_e[TG_FULL * 128:].rearrange("(x p) -> x p", x=1))
            psr = mm_tile(TG_REM, 1)
            T(psr, erem[:], ident[:1, :1])
            nc.vector.tensor_copy(eT[:TG_REM, TG_FULL:NSUB], psr)

        # Chebyshev recurrence for cos/sin(k*e/pi)
        s1 = wtile([128, NSUB], f32, "s1")
        nc.scalar.activation(s1[:], eT[:], AF.Sin, scale=1.0 / math.pi)
        c1 = wtile([128, NSUB], f32, "c1")
        nc.vector.tensor_mul(c1[:], s1[:], s1[:])
        nc.scalar.activation(c1[:], c1[:], AF.Sqrt, scale=-1.0, bias=1.0)
        twoc = wtile([128, NSUB], f32, "twoc")
        nc.vector.tensor_add(twoc[:], c1[:], c1[:])

        phi = wtile([128, NSUB], f32, "phi")
        ktmp = wtile([128, NSUB], f32, "ktmp")
        nc.vector.tensor_scalar(phi[:], c1[:], scalar1=sB[:, 8:9], scalar2=None, op0=ALU.mult)
        nc.vector.tensor_scalar(ktmp[:], s1[:], scalar1=sB[:, 18:19],
                                scalar2=None, op0=ALU.mult)
        nc.vector.tensor_add(phi[:], phi[:], ktmp[:])
        cp, sp = c1, s1
        cpp, spp = None, None
        for k in range(2, 11):
            ck = rtile([128, NSUB], f32, "ckt")
            sk = rtile([128, NSUB], f32, "skt")
            nc.vector.tensor_mul(ck[:], twoc[:], cp[:])
            nc.vector.tensor_mul(sk[:], twoc[:], sp[:])
            if k == 2:
                nc.vector.tensor_scalar(ck[:], ck[:], scalar1=one_col[:],
                                        scalar2=None, op0=ALU.subtract)
            else:
                nc.vector.tensor_sub(ck[:], ck[:], cpp[:])
                nc.vector.tensor_sub(sk[:], sk[:], spp[:])
            nc.vector.tensor_scalar(ktmp[:], ck[:],
                                    scalar1=sB[:, 7 + k:8 + k], scalar2=None, op0=ALU.mult)
            nc.vector.tensor_add(phi[:], phi[:], ktmp[:])
            nc.vector.tensor_scalar(ktmp[:], sk[:],
                                    scalar1=sB[:, 17 + k:18 + k], scalar2=None, op0=ALU.mult)
            nc.vector.tensor_add(phi[:], phi[:], ktmp[:])
            cpp, spp = cp, sp
            cp, sp = ck, sk
        ne = wtile([128, NSUB], f32, "ne")
        nc.vector.tensor_scalar(ne[:], phi[:], scalar1=kbias_c, op0=ALU.add,
                                scalar2=alpha_c, op1=ALU.mult)

        # ---------- folded weights (LN affine into projections) ----------
        def fold_w(name, w_sb, g_col):
            t = wtile([128, D], bf16, name)
            nc.vector.tensor_scalar(t[:], w_sb[:], scalar1=g_col[:], scalar2=None, op0=ALU.mult)
            return t

        Wk1b = fold_w("Wk1b", k1w, mg_c); Wk2b = fold_w("Wk2b", k2w, mg_c)
        Wvb = fold_w("Wvb", vw, mg_c)
        Wq1 = wtile([128, D], f32, "Wq1")
        nc.vector.tensor_scalar(Wq1[:], q1w[:], scalar1=mg_c[:], scalar2=None, op0=ALU.mult)
        Wq2 = wtile([128, D], f32, "Wq2")
        nc.vector.tensor_scalar(Wq2[:], q2w[:], scalar1=mg_c[:], scalar2=None, op0=ALU.mult)
        W1pb = fold_w("W1pb", f1w, fg_c)
        Wob = wtile([128, D], bf16, "Wob")
        nc.vector.tensor_scalar(Wob[:], ow[:], scalar1=ag_c[:], op0=ALU.mult,
                                scalar2=c08_col[:], op1=ALU.mult)

        def fold_b(name, w_sb, beta_col, b_row):
            # row [1, D] bias: beta^T @ W + b
            psb = mm_tile(1, D)
            nc.tensor.matmul(psb, beta_col[:], w_sb[:])
            t = wtile([1, D], f32, name)
            nc.vector.tensor_add(t[:], psb, b_row[:])
            return t

        bk1_r = fold_b("bk1_r", k1w, mb_c, k1b_r)
        bk2_r = fold_b("bk2_r", k2w, mb_c, k2b_r)
        bv_r = fold_b("bv_r", vw, mb_c, vb_r)
        psq = mm_tile(128, 1)
        nc.tensor.matmul(psq, q1w[:], mb_c[:])
        bq1_c = wtile([128, 1], f32, "bq1_c")
        nc.vector.tensor_add(bq1_c[:], psq, q1b_c[:])
        psq2 = mm_tile(128, 1)
        nc.tensor.matmul(psq2, q2w[:], mb_c[:])
        bq2_c = wtile([128, 1], f32, "bq2_c")
        nc.vector.tensor_add(bq2_c[:], psq2, q2b_c[:])
        # column biases for transposed epilogue
        pso = mm_tile(128, 1)
        nc.tensor.matmul(pso, ow[:], ab_c[:])
        bo_c = wtile([128, 1], f32, "bo_c")
        nc.vector.tensor_scalar(bo_c[:], pso, scalar1=c08_col[:], scalar2=None, op0=ALU.mult)
        nc.vector.tensor_add(bo_c[:], bo_c[:], ob_c[:])
        psp1 = mm_tile(128, 1)
        nc.tensor.matmul(psp1, f1w[:], fb_c[:])
        b1p_c = wtile([128, 1], f32, "b1p_c")
        nc.vector.tensor_add(b1p_c[:], psp1, f1b_c[:])

        def bcast_row(name, row_sb):
            psb = mm_tile(128, D)
            nc.tensor.matmul(psb, ones_row[:], row_sb[:])
            t = wtile([128, D], f32, name)
            nc.vector.tensor_copy(t[:], psb)
            return t

        bk1_B = bcast_row("bk1_B", bk1_r)
        bk2_B = bcast_row("bk2_B", bk2_r)
        bv_B = bcast_row("bv_B", bv_r)

        # ---------- DRAM staging ----------
        p1_in, p1_out = [], []
        for c, (co, cw) in enumerate(CHUNKS):
            p1_in.append(dram.tile([128, cw], bf16, tag=f"p1in{c}",
                                   name=f"p1in{c}"))
            p1_out.append(dram.tile([128, cw], bf16, tag=f"p1out{c}",
                                    name=f"p1out{c}", addr_space=shared_space))
        gr_in = dram.tile([128, 2 * D], f32, tag="gr_in", name="gr_in")
        gr_out = dram.tile([128, 2 * D], f32, tag="gr_out", name="gr_out",
                           addr_space=shared_space)

        # ---------- u streaming loads (HWDGE fp32, full rate) ----------
        u_tiles = {}

        def emit_u_loads(c):
            co, cw = CHUNKS[c]
            tiles = {}
            for b, (bo, bw) in enumerate(_splits(cw, 512)):
                for r, (ro, rw) in enumerate(ROWS):
                    ut = ustream.tile([128, 512], bf16, tag="u",
                                      name=f"u{c}_{r}_{b}")[:rw, :bw]
                    eng = nc.scalar if (r + b) % 2 == 0 else nc.sync
                    eng.dma_start(
                        out=ut, in_=t_u[ro:ro + rw, co + bo:co + bo + bw])
                    tiles[(r, b)] = ut
            u_tiles[c] = tiles

        emit_u_loads(0)

        # ================= phase A: feature encoder =================
        h1Tb = wtile([128, NLOC], bf16, "h1Tb")
        for go, gw in _splits(NLOC, 512):
            xTg = rowtmp.tile([128, KX, 512], bf16, tag="xTg", bufs=2,
                              name=f"xTg_{nc.next_id()}")
            for ro, rw in _splits(gw, 128):
                xt = rowtmp.tile([128, NF], f32, tag="xt", bufs=2,
                                 name=f"xt_{nc.next_id()}")[:rw]
                nc.sync.dma_start(out=xt, in_=t_x[go + ro:go + ro + rw, :])
                xb16 = rowtmp.tile([128, NF], bf16, tag="xb16", bufs=2,
                                   name=f"xb16_{nc.next_id()}")[:rw]
                nc.vector.tensor_copy(xb16, xt)
                for kt in range(KX):
                    pst = tb_tile(128, rw)
                    T(pst, xb16[:, kt * 128:(kt + 1) * 128],
                      identb[:rw, :rw])
                    nc.vector.tensor_copy(xTg[:, kt, ro:ro + rw], pst)
            psh1 = p2_tile(gw)
            for kt in range(KX):
                nc.tensor.matmul(psh1, few1b[:, kt, :], xTg[:, kt, :gw],
                                 start=(kt == 0), stop=(kt == KX - 1))
            nc.scalar.activation(h1Tb[:, go:go + gw], psh1, AF.Relu,
                                 bias=feb1_c[:])

        hT = wtile([128, NLOC], f32, "hT")
        for bo, bw in BLK:
            psh = p2_tile(bw)
            nc.tensor.matmul(psh, few2b[:], h1Tb[:, bo:bo + bw])
            nc.vector.tensor_scalar(hT[:, bo:bo + bw], psh, scalar1=feb2_c[:],
                                    scalar2=None, op0=ALU.add)

        def dbg_dump(dst_dram, src_ap, width, off=0):
            nc.sync.dma_start(out=dst_dram[:, off:off + width],
                              in_=src_ap[:, :width])

        if debug:
            dbg_dump(t_dhT, hT, NLOC)

        # ---- per-row-tile: h16 rows, LN, hnT, k/v projections, gram ----
        def layer_norm(src_ap, rw, out_ap):
            stats = rtile([128, 6], f32, "stats")
            nc.vector.bn_stats(stats[:rw], src_ap)
            mv = rtile([128, 2], f32, "mv")
            nc.vector.bn_aggr(mv[:rw], stats[:rw])
            rs = rtile([128, 1], f32, "rs")
            nc.scalar.activation(rs[:rw], mv[:rw, 1:2], AF.Sqrt,
                                 bias=eps_col[:rw])
            nc.vector.reciprocal(rs[:rw], rs[:rw])
            nc.vector.tensor_scalar(out_ap, src_ap, scalar1=mv[:rw, 0:1],
                                    op0=ALU.subtract, scalar2=rs[:rw],
                                    op1=ALU.mult)

        h16 = wtile([128, NT, D], bf16, "h16")
        hnTb = wtile([128, NLOC], bf16, "hnTb")
        gram = wtile([128, 2 * D], f32, "gram")
        for r, (ro, rw) in enumerate(ROWS):
            psr = mm_tile(rw, D)
            nc.tensor.matmul(psr, h1Tb[:, ro:ro + rw], few2b[:])
            hrow = rtile([128, D], f32, "hrow")[:rw]
            nc.vector.tensor_add(hrow, psr, feb2_B[:rw])
            nc.vector.tensor_add(h16[:rw, r, :], psr, feb2_B[:rw])
            hn = rtile([128, D], f32, "hn")[:rw]
            layer_norm(hrow, rw, hn)
            hn16 = rtile([128, D], bf16, "hn16")[:rw]
            nc.vector.tensor_copy(hn16, hn)
            psT = tb_tile(128, rw)
            T(psT, hn16, identb[:rw, :rw])
            nc.vector.tensor_copy(hnTb[:, ro:ro + rw], psT)
            k12t = rtile([128, 2, D], bf16, "k12t")
            vt = rtile([128, D], bf16, "vt")
            for dst, W, bB in ((k12t[:rw, 0, :], Wk1b, bk1_B),
                               (k12t[:rw, 1, :], Wk2b, bk2_B),
                               (vt[:rw], Wvb, bv_B)):
                psp = mm_tile(rw, D)
                nc.tensor.matmul(psp, hnTb[:, ro:ro + rw], W[:])
                nc.vector.tensor_add(dst, psp, bB[:rw])
            # gram transposed:  psg = v^T @ [k1 | k2], accumulated in SBUF
            psg = mm_tile(128, 2 * D)
            nc.tensor.matmul(psg, vt[:rw], k12t[:rw, :, :])
            if r == 0:
                nc.vector.tensor_copy(gram[:], psg)
            else:
                nc.vector.tensor_add(gram[:], gram[:], psg)

        if debug:
            dbg_dump(t_dhnT, hnTb, NLOC)
        nc.gpsimd.dma_start(out=gr_in[:], in_=gram[:])
        nc.gpsimd.collective_compute("AllReduce", ALU.add, replica_groups=rg,
                                     ins=[gr_in.opt()], outs=[gr_out.opt()])

        # ---------- transposed-layout LayerNorm helper ----------
        def lnT(x_sb, out_bf, pfx):
            xb = wpool.tile([128, NLOC], bf16, tag="ln_xb",
                            name=f"{pfx}_xb")
            nc.vector.tensor_copy(xb[:], x_sb[:])
            x2b = wpool.tile([128, NLOC], bf16, tag="ln_x2b",
                             name=f"{pfx}_x2b")
            nc.vector.tensor_mul(x2b[:], x_sb[:], x_sb[:])
            for bo, bw in BLK:
                def row(tag, dt_):
                    return rowtmp.tile([1, 512], dt_, tag=tag, bufs=2,
                                       name=f"{tag}_{nc.next_id()}")[:, :bw]
                psm = mm_tile(1, bw)
                nc.tensor.matmul(psm, oinv_col_b[:], xb[:, bo:bo + bw])
                mrow = row("ln_m", f32)
                nc.vector.tensor_copy(mrow, psm)
                psq_ = mm_tile(1, bw)
                nc.tensor.matmul(psq_, oinv_col_b[:], x2b[:, bo:bo + bw])
                vrow = row("ln_v", f32)
                nc.vector.tensor_mul(vrow, mrow, mrow)
                nc.vector.tensor_sub(vrow, psq_, vrow)           # var
                nc.scalar.activation(vrow, vrow, AF.Sqrt, bias=eps_col[:1])
                nc.vector.reciprocal(vrow, vrow)                 # rs
                m_b = row("ln_mb", bf16)
                nc.vector.tensor_copy(m_b, mrow)
                rs_b = row("ln_rb", bf16)
                nc.vector.tensor_copy(rs_b, vrow)
                psM = mm_tile(128, bw)
                nc.tensor.matmul(psM, ones_row_b[:], m_b)
                psR = mm_tile(128, bw)
                nc.tensor.matmul(psR, ones_row_b[:], rs_b)
                dtmp = rowtmp.tile([128, 512], f32, tag="btmp", bufs=2,
                                   name=f"lnd_{nc.next_id()}")[:, :bw]
                nc.vector.tensor_sub(dtmp, x_sb[:, bo:bo + bw], psM)
                nc.vector.tensor_mul(out_bf[:, bo:bo + bw], dtmp, psR)

        # ================= chunk pipeline =================
        henc = wtile([128, NLOC], f32, "henc")
        haT = wtile([128, NLOC], f32, "haT")
        sT = wtile([128, NLOC], f32, "sT")
        aTb = wtile([128, NLOC], bf16, "aTb")

        def emit_pass1(c):
            co, cw = CHUNKS[c]
            blocks = _splits(cw, 512)
            ut = u_tiles.pop(c)
            p1sb = p1sbp.tile([128, CHUNK], bf16, tag="p1sb",
                              name=f"p1sb{c}")[:, :cw]
            for b, (bo, bw) in enumerate(blocks):
                ps1 = p1_tile(bw)
                for r, (ro, rw) in enumerate(ROWS):
                    nc.tensor.matmul(ps1, h16[:rw, r, :], ut[(r, b)],
                                     start=(r == 0), stop=(r == NT - 1))
                nc.vector.tensor_copy(p1sb[:, bo:bo + bw], ps1)
            nc.gpsimd.dma_start(out=p1_in[c][:], in_=p1sb)
            nc.gpsimd.collective_compute(
                "AllReduce", ALU.add, replica_groups=rg,
                ins=[p1_in[c].opt()], outs=[p1_out[c].opt()])
            if debug and c == 0:
                dbg_dump(t_dp1, p1sb, cw)

        def emit_pass2(c):
            co, cw = CHUNKS[c]
            subs = _splits(cw, 128)
            zc = zcp.tile([128, CHUNK], bf16, tag="zc",
                          name=f"zc_{c}")[:, :cw]
            nc.scalar.dma_start(out=zc, in_=p1_out[c][:])
            z16 = z16p.tile([128, NSUB_C, D], bf16, tag="z16", name=f"z16_{c}")
            for t, (so, sw) in enumerate(subs):
                psz = ps_t.tile([128, 128], bf16, tag="pstb",
                                name=f"psz_{nc.next_id()}")[:sw, :]
                T(psz, zc[:, so:so + sw], identb[:, :])
                gidx = (co + so) // 128
                nc.vector.tensor_scalar(z16[:sw, t, :], psz,
                                        scalar1=ne[:sw, gidx:gidx + 1],
                                        scalar2=None, op0=ALU.mult)
            uTc = []
            for t, (so, sw) in enumerate(subs):
                uTt = uTp.tile([128, NLOC], bf16, tag="uT",
                               name=f"uTl{c}_{t}")[:sw]
                eng = nc.sync if t % 2 == 0 else nc.scalar
                eng.dma_start(out=uTt, in_=t_uT[co + so:co + so + sw, :])
                uTc.append(uTt)
            if debug and c == 0:
                dbg_dump(t_duT, uTc[0], NLOC)
                for t, (so, sw) in enumerate(subs):
                    dbg_dump(t_dz, z16[:, t, :], D, off=t * D)
            if debug:
                dbg_dump(t_duTall, uTc[0], NLOC, off=c * NLOC_PAD)
                dbg_dump(t_dzall, z16[:, 0, :], D, off=c * D)
            ps2 = [p2_tile(iw) for io, iw in IBLK]
            for t, (so, sw) in enumerate(subs):
                for ib, (io, iw) in enumerate(IBLK):
                    nc.tensor.matmul(ps2[ib], z16[:sw, t, :],
                                     uTc[t][:sw, io:io + iw],
                                     start=(t == 0),
                                     stop=(t == len(subs) - 1))
            for ib, (io, iw) in enumerate(IBLK):
                if c == 0:
                    nc.vector.tensor_copy(henc[:, io:io + iw], ps2[ib])
                else:
                    nc.vector.tensor_add(henc[:, io:io + iw],
                                         henc[:, io:io + iw], ps2[ib])

        def emit_watt():
            # kv holds (v^T k1 | v^T k2) = (k1v^T | k2v^T)
            kv = wtile([128, 2 * D], f32, "kv")
            nc.scalar.dma_start(out=kv[:], in_=gr_out[:])
            psk1 = mm_tile(128, 128)
            T(psk1, kv[:, :D], ident[:])
            k1vs = wtile([128, D], f32, "k1vs")
            nc.vector.tensor_copy(k1vs[:], psk1)
            psk2 = mm_tile(128, 128)
            T(psk2, kv[:, D:], ident[:])
            k2vs = wtile([128, D], f32, "k2vs")
            nc.vector.tensor_copy(k2vs[:], psk2)
            psq1T = mm_tile(128, 128)
            T(psq1T, Wq1[:], ident[:])
            Wq1T = wtile([128, D], f32, "Wq1T")
            nc.vector.tensor_copy(Wq1T[:], psq1T)
            psq2T = mm_tile(128, 128)
            T(psq2T, Wq2[:], ident[:])
            Wq2T = wtile([128, D], f32, "Wq2T")
            nc.vector.tensor_copy(Wq2T[:], psq2T)

            Watt = wtile([128, D], f32, "Watt")
            ps_w2e = mm_tile(D, D)
            nc.tensor.matmul(ps_w2e, Wq2T[:], k2vs[:])
            nc.vector.tensor_scalar(Watt[:], ps_w2e, scalar1=neglam_c,
                                    scalar2=None, op0=ALU.mult)
            ps_w1e = mm_tile(D, D)
            nc.tensor.matmul(ps_w1e, Wq1T[:], k1vs[:])
            nc.vector.tensor_add(Watt[:], Watt[:], ps_w1e)
            Wattb = wtile([128, D], bf16, "Wattb")
            nc.vector.tensor_copy(Wattb[:], Watt[:])

            batt_c = wtile([128, 1], f32, "batt_c")
            ps_b2 = mm_tile(128, 1)
            nc.tensor.matmul(ps_b2, k2vs[:], bq2_c[:])
            nc.vector.tensor_scalar(batt_c[:], ps_b2, scalar1=neglam_c,
                                    scalar2=None, op0=ALU.mult)
            ps_b1 = mm_tile(128, 1)
            nc.tensor.matmul(ps_b1, k1vs[:], bq1_c[:])
            nc.vector.tensor_add(batt_c[:], batt_c[:], ps_b1)
            return Wattb, batt_c

        for c in range(NCH):
            if c + 1 < NCH:
                emit_u_loads(c + 1)
            emit_pass1(c)
            if c >= DEPTH:
                emit_pass2(c - DEPTH)
            if c == 1:
                Wattb, batt_c = emit_watt()
            if c == 2:
                # sT = Watt^T @ hnT + batt  (transposed layout)
                for bo, bw in BLK:
                    pss = mm_tile(128, bw)
                    nc.tensor.matmul(pss, Wattb[:], hnTb[:, bo:bo + bw])
                    nc.vector.tensor_scalar(sT[:, bo:bo + bw], pss,
                                            scalar1=batt_c[:], scalar2=None,
                                            op0=ALU.add)
                lnT(sT, aTb, "s")
            if c == 3:
                # haT = hT + Wo'^T @ aT + bo
                for bo, bw in BLK:
                    psa = mm_tile(128, bw)
                    nc.tensor.matmul(psa, Wob[:], aTb[:, bo:bo + bw])
                    atmp = rowtmp.tile([128, 512], f32, tag="btmp", bufs=2,
                                       name=f"atmp_{nc.next_id()}")[:, :bw]
                    nc.vector.tensor_scalar(atmp, psa, scalar1=bo_c[:],
                                            scalar2=None, op0=ALU.add)
                    nc.vector.tensor_add(haT[:, bo:bo + bw],
                                         hT[:, bo:bo + bw], atmp)
                if debug:
                    dbg_dump(t_dhaT, haT, NLOC)
        for c in range(NCH - DEPTH, NCH):
            emit_pass2(c)

        # ================= transposed epilogue =================
        if debug:
            dbg_dump(t_dhenc, henc, NLOC)
        nc.vector.tensor_add(haT[:], haT[:], henc[:])            # mhT
        fTb = wtile([128, NLOC], bf16, "fTb")
        lnT(haT, fTb, "f")
        if debug:
            dbg_dump(t_dfT, fTb, NLOC)
        gb = wtile([128, NLOC], bf16, "gb")
        for bo, bw in BLK:
            psg_ = mm_tile(128, bw)
            nc.tensor.matmul(psg_, W1pb[:], fTb[:, bo:bo + bw])
            nc.scalar.activation(gb[:, bo:bo + bw], psg_, AF.Gelu,
                                 bias=b1p_c[:])
        outT = sT  # sT is dead after lnT(sT); reuse its buffer
        for bo, bw in BLK:
            pso_ = mm_tile(128, bw)
            nc.tensor.matmul(pso_, f2wb[:], gb[:, bo:bo + bw])
            otmp = rowtmp.tile([128, 512], f32, tag="btmp", bufs=2,
                               name=f"otmp_{nc.next_id()}")[:, :bw]
            nc.vector.tensor_scalar(otmp, pso_, scalar1=f2b_c[:],
                                    scalar2=None, op0=ALU.add)
            nc.vector.tensor_add(outT[:, bo:bo + bw],
                                 haT[:, bo:bo + bw], otmp)
        nc.sync.dma_start(out=t_out[:], in_=outT[:])

    nc.compile()
    return nc


# ==================== host-side entry point ====================

_CACHED = {}


def _get_nc(N=N_FULL, NF=NF_FULL, CORES=CORES_FULL, CHUNK=CHUNK_FULL):
    key = (N, NF, CORES, CHUNK)
    if key not in _CACHED:
        _CACHED[key] = build_kernel(N, NF, CORES, CHUNK)
    return _CACHED[key]


def make_in_maps(inputs, N, CORES):
    import ml_dtypes

    NLOC = N // CORES
    bf = ml_dtypes.bfloat16
    full = {k: np.ascontiguousarray(np.asarray(v, dtype=np.float32))
            for k, v in inputs.items()}
    in_maps = []
    for c in range(CORES):
        m = {}
        for k, v in full.items():
            if k == "x":
                m[k] = np.ascontiguousarray(v[c * NLOC:(c + 1) * NLOC])
            elif k == "u":
                sh = v[c * NLOC:(c + 1) * NLOC]
                m["u"] = np.ascontiguousarray(sh.astype(bf))
                m["uT"] = np.ascontiguousarray(sh.T.astype(bf))
            else:
                m[k] = v
        in_maps.append(m)
    return in_maps


def assemble_out(res, CORES=CORES_FULL):
    # per-core outputs are [D, NLOC] (transposed); transpose + concat rows
    return np.concatenate(
        [np.asarray(res.results[c]["out"]).T for c in range(CORES)],
        axis=0).astype(np.float32)


def kernel(**inputs):
    from concourse import bass_utils

    nc = _get_nc()
    in_maps = make_in_maps(inputs, N_FULL, CORES_FULL)
    res = bass_utils.run_bass_kernel_spmd(nc, in_maps,
                                          core_ids=list(range(CORES_FULL)))
    return assemble_out(res)


if __name__ == "__main__":
    build_kernel()
    print("build ok")



# revision 14
# speedup vs baseline: 1.1838x; 1.1838x over previous
"""Trainium2 Bass kernel for nn_NoFoDifformer_FourierKAN (8-core SPMD), v3.

Sharding: u and nodes row-wise across 8 cores (1250 rows each). The z = u^T h
partial sums are all-reduced per column-chunk (bf16); the [d,d] Gram matrix of
the normalized activations rides inside the first chunk's all-reduce. Small
weights are pre-folded and packed on the HOST (LayerNorm affines folded into
projections, lambda/\phi(e) evaluated in numpy) so the device preamble is just
a handful of panel DMAs. Per-core outputs are [d, n_loc] (transposed) and
transposed+concatenated on the host.

Key structure (v3):
- host passes xT/u/uT as bf16 shards + packed weight panels; the entire
  activation path runs in transposed layout [d, i] with weight-stationary
  matmuls; LayerNorm stats via ones-matmul partition sums (Rsqrt on Scalar).
- k1^T v / k2^T v come from the Gram trick: G = hn^T hn and s = hn^T 1 are
  all-reduced (merged into chunk-0's AR) and the [d,d] products are formed
  post-AR with a few tiny matmuls -- no per-row k/q/v projections.
- pass1 streams u in [rw,1024] quarter tiles (few big DMAs, short lifetime,
  2 PSUM banks); pass2 reads back the AR'd z, transposes 128-subtiles on PE,
  scales by the host-computed spectral filter ne, and contracts with
  prefetched uT tiles.
"""

import numpy as np

N_FULL = 10000
NF_FULL = 512
D = 128
CORES_FULL = 8
CHUNK_FULL = 2048
LAMBDA_INIT = 0.2
GEXT = 256  # extra AR columns on chunk 0 for (G | s)


def _splits(total, step):
    return [(o, min(step, total - o)) for o in range(0, total, step)]


def build_kernel(N=N_FULL, NF=NF_FULL, CORES=CORES_FULL, CHUNK=CHUNK_FULL):
    import concourse.bacc as bacc
    import concourse.tile as tile
    from concourse import mybir
    from concourse.masks import make_identity
    from contextlib import ExitStack

    dt = mybir.dt
    f32 = dt.float32
    bf16 = dt.bfloat16
    AF = mybir.ActivationFunctionType
    ALU = mybir.AluOpType

    NLOC = N // CORES                   # 1250
    ROWS = _splits(NLOC, 128)           # 9x128 + 98
    NT = len(ROWS)
    KX = NF // 128
    CHUNKS = _splits(N, CHUNK)          # 5 chunks
    NCH = len(CHUNKS)
    assert NCH == 5, "pipeline schedule below is written for 5 chunks"
    NSUB = (N + 127) // 128             # 79
    BLK = _splits(NLOC, 512)            # [d, NLOC] op blocks
    rg = [list(range(CORES))]
    shared_space = "Shared" if CORES > 4 else "Local"

    nc = bacc.Bacc("TRN2", target_bir_lowering=False, debug=False,
                   num_devices=CORES)

    # ---------------- DRAM I/O ----------------
    t_u = nc.dram_tensor("u", [NLOC, N], bf16, kind="ExternalInput")
    t_uT = nc.dram_tensor("uT", [N, NLOC], bf16, kind="ExternalInput")
    t_xT = nc.dram_tensor("xT", [NF, NLOC], bf16, kind="ExternalInput")
    t_wb = nc.dram_tensor("wb", [128, 13 * 128], bf16, kind="ExternalInput")
    t_colw = nc.dram_tensor("colw", [128, 8], f32, kind="ExternalInput")
    t_colb = nc.dram_tensor("colb", [128, 2], bf16, kind="ExternalInput")
    t_rowb = nc.dram_tensor("rowb", [1, 4 * 128], bf16, kind="ExternalInput")
    t_roww = nc.dram_tensor("roww", [1, 128], f32, kind="ExternalInput")
    t_ne = nc.dram_tensor("ne", [128, NSUB], f32, kind="ExternalInput")
    t_out = nc.dram_tensor("out", [D, NLOC], f32, kind="ExternalOutput")

    with tile.TileContext(nc) as tc, ExitStack() as ctx:
        wpool = ctx.enter_context(tc.tile_pool(name="wpool", bufs=1))
        rowtmp = ctx.enter_context(tc.tile_pool(name="rowtmp", bufs=3))
        ustream = ctx.enter_context(tc.tile_pool(name="ustream", bufs=8))
        uTp = ctx.enter_context(tc.tile_pool(name="uTp", bufs=20))
        zcp = ctx.enter_context(tc.tile_pool(name="zcp", bufs=2))
        z16p = ctx.enter_context(tc.tile_pool(name="z16p", bufs=2))
        p1sbp = ctx.enter_context(tc.tile_pool(name="p1sbp", bufs=2))
        dram = ctx.enter_context(tc.tile_pool(name="dram", bufs=1, space="DRAM"))
        ps_p1 = ctx.enter_context(tc.tile_pool(name="ps_p1", bufs=2, space="PSUM"))
        ps_p2 = ctx.enter_context(tc.tile_pool(name="ps_p2", bufs=3, space="PSUM"))
        ps_mm = ctx.enter_context(tc.tile_pool(name="ps_mm", bufs=1, space="PSUM"))
        ps_t = ctx.enter_context(tc.tile_pool(name="ps_t", bufs=2, space="PSUM"))

        def p1_tile(w):
            return ps_p1.tile([128, 512], f32, tag="p1",
                              name=f"p1_{nc.next_id()}")[:, :w]

        def p2_tile(w):
            return ps_p2.tile([128, 512], f32, tag="p2",
                              name=f"p2_{nc.next_id()}")[:, :w]

        def mm_tile(p, w):
            return ps_mm.tile([128, 512], f32, tag="mmp",
                              name=f"mm_{nc.next_id()}")[:p, :w]

        def tb_tile(p, w):
            return ps_t.tile([128, 128], bf16, tag="pstb",
                             name=f"pstb_{nc.next_id()}")[:p, :w]

        def wtile(shape, dtype, name):
            return wpool.tile(shape, dtype, tag=name, name=name)

        def rtile(shape, dtype, tag):
            return rowtmp.tile(shape, dtype, tag=tag,
                               name=f"{tag}_{nc.next_id()}")

        def T(out_psum, in_sbuf, identity):
            nc.tensor.matmul(out_psum, in_sbuf, identity, is_transpose=True)

        # ================= constants & weights =================
        identb = wtile([128, 128], bf16, "identb")
        make_identity(nc, identb[:])
        ones_row_b = wtile([1, 128], bf16, "ones_row_b")
        nc.vector.memset(ones_row_b[:], 1.0)
        ones_col_b = wtile([128, 1], bf16, "ones_col_b")
        nc.vector.memset(ones_col_b[:], 1.0)
        oinv_col_b = wtile([128, 1], bf16, "oinv_col_b")
        nc.vector.memset(oinv_col_b[:], 1.0 / 128.0)
        eps_col = wtile([128, 1], f32, "eps_col")
        nc.vector.memset(eps_col[:], 1e-5)

        wb = wtile([128, 13 * 128], bf16, "wb")
        nc.gpsimd.dma_start(out=wb[:], in_=t_wb[:])
        colw = wtile([128, 8], f32, "colw")
        nc.gpsimd.dma_start(out=colw[:], in_=t_colw[:])
        colb = wtile([128, 2], bf16, "colb")
        nc.gpsimd.dma_start(out=colb[:], in_=t_colb[:])
        rowb = wtile([1, 4 * 128], bf16, "rowb")
        nc.gpsimd.dma_start(out=rowb[:], in_=t_rowb[:])
        roww = wtile([1, 128], f32, "roww")
        nc.gpsimd.dma_start(out=roww[:], in_=t_roww[:])
        ne = wtile([128, NSUB], f32, "ne")
        nc.gpsimd.dma_start(out=ne[:], in_=t_ne[:])

        def P(i):  # weight panel i of wb
            return wb[:, i * 128:(i + 1) * 128]
        few2b = P(4)
        Wk1b, Wk2b, Wvb = P(5), P(6), P(7)
        Wq1Tb, Wq2Tsb = P(8), P(9)
        Wob, W1pb, f2wb = P(10), P(11), P(12)
        feb1_c = colw[:, 0:1]
        feb2_c = colw[:, 1:2]
        bo_c = colw[:, 2:3]
        b1p_c = colw[:, 3:4]
        f2b_c = colw[:, 4:5]

        # xT into SBUF (4 partition k-tiles)
        xT4 = wtile([128, KX, NLOC], bf16, "xT4")
        for kt in range(KX):
            nc.sync.dma_start(out=xT4[:, kt, :],
                              in_=t_xT[kt * 128:(kt + 1) * 128, :])

        # ---------- u streaming loads (quarter tiles, sync queue) ----------
        u_tiles = {}

        def emit_u_loads(c):
            co, cw = CHUNKS[c]
            tiles = {}
            for q, (qo, qw) in enumerate(_splits(cw, 1024)):
                for r, (ro, rw) in enumerate(ROWS):
                    ut = ustream.tile([128, 1024], bf16, tag="u",
                                      name=f"u{c}_{q}_{r}")[:rw, :qw]
                    nc.sync.dma_start(
                        out=ut, in_=t_u[ro:ro + rw, co + qo:co + qo + qw])
                    tiles[(q, r)] = ut
            u_tiles[c] = tiles

        emit_u_loads(0)

        # ================= phase A: feature encoder (transposed) ==========
        hT = wtile([128, NLOC], f32, "hT")
        hTb = wtile([128, NLOC], bf16, "hTb")
        h16 = wtile([128, NT, D], bf16, "h16")
        for go, gw in BLK:
            psh1 = p2_tile(gw)
            for kt in range(KX):
                nc.tensor.matmul(psh1, P(kt), xT4[:, kt, go:go + gw],
                                 start=(kt == 0), stop=(kt == KX - 1))
            h1t = rtile([128, 512], bf16, "h1t")[:, :gw]
            nc.scalar.activation(h1t, psh1, AF.Relu, bias=feb1_c[:])
            pshT = p2_tile(gw)
            nc.tensor.matmul(pshT, few2b, h1t)
            nc.vector.tensor_scalar(hT[:, go:go + gw], pshT,
                                    scalar1=feb2_c, scalar2=None, op0=ALU.add)
            nc.scalar.activation(hTb[:, go:go + gw], hT[:, go:go + gw],
                                 AF.Copy)
        for r, (ro, rw) in enumerate(ROWS):
            pst = tb_tile(rw, 128)
            T(pst, hTb[:, ro:ro + rw], identb[:])
            nc.vector.tensor_copy(h16[:rw, r, :], pst)

        # ---------- transposed-layout LayerNorm helper ----------
        def lnT(x_sb, out_bf, pfx, xb=None):
            if xb is None:
                xb = wpool.tile([128, NLOC], bf16, tag="ln_xb",
                                name=f"{pfx}_xb")
                nc.scalar.activation(xb[:], x_sb[:], AF.Copy)
            x2b = wpool.tile([128, NLOC], bf16, tag="ln_x2b",
                             name=f"{pfx}_x2b")
            nc.vector.tensor_mul(x2b[:], x_sb[:], x_sb[:])

            def frow(tag, dt_):
                return rowtmp.tile([1, NLOC], dt_, tag=tag, bufs=2,
                                   name=f"{tag}_{nc.next_id()}")
            mrow = frow("ln_m", f32)
            rsrow = frow("ln_r", f32)
            for bo, bw in BLK:
                psm = mm_tile(1, bw)
                nc.tensor.matmul(psm, oinv_col_b[:], xb[:, bo:bo + bw])
                nc.vector.tensor_copy(mrow[:, bo:bo + bw], psm)
                psq_ = mm_tile(1, bw)
                nc.tensor.matmul(psq_, oinv_col_b[:], x2b[:, bo:bo + bw])
                nc.vector.tensor_mul(rsrow[:, bo:bo + bw],
                                     mrow[:, bo:bo + bw], mrow[:, bo:bo + bw])
                nc.vector.tensor_sub(rsrow[:, bo:bo + bw], psq_,
                                     rsrow[:, bo:bo + bw])         # var
            nc.scalar.activation(rsrow[:], rsrow[:], AF.Sqrt,
                                 bias=eps_col[:1])
            nc.vector.reciprocal(rsrow[:], rsrow[:])               # 1/sqrt
            m_b = frow("ln_mb", bf16)
            nc.vector.tensor_copy(m_b[:], mrow[:])
            rs_b = frow("ln_rb", bf16)
            nc.vector.tensor_copy(rs_b[:], rsrow[:])
            for bo, bw in BLK:
                psM = p2_tile(bw)
                nc.tensor.matmul(psM, ones_row_b[:], m_b[:, bo:bo + bw])
                psR = p2_tile(bw)
                nc.tensor.matmul(psR, ones_row_b[:], rs_b[:, bo:bo + bw])
                dtmp = rowtmp.tile([128, 512], f32, tag="btmp", bufs=2,
                                   name=f"lnd_{nc.next_id()}")[:, :bw]
                nc.vector.tensor_sub(dtmp, x_sb[:, bo:bo + bw], psM)
                nc.vector.tensor_mul(out_bf[:, bo:bo + bw], dtmp, psR)

        hnTb = wtile([128, NLOC], bf16, "hnTb")
        lnT(hT, hnTb, "hn", xb=hTb)

        # ---------- Gram: G = hn^T hn, s = hn^T 1 (into gram_sb bf16) ------
        gram_sb = wtile([128, GEXT], bf16, "gram_sb")
        psGS = ps_mm.tile([128, 512], f32, tag="mmp", name="psGS")
        for r, (ro, rw) in enumerate(ROWS):
            pst = tb_tile(rw, 128)
            T(pst, hnTb[:, ro:ro + rw], identb[:])
            hn_r = rtile([128, 128], bf16, "hn_r")[:rw]
            nc.vector.tensor_copy(hn_r, pst)
            nc.tensor.matmul(psGS[:, 0:128], hn_r, hn_r,
                             start=(r == 0), stop=(r == NT - 1))
            nc.tensor.matmul(psGS[:1, 128:256], ones_col_b[:rw], hn_r,
                             start=(r == 0), stop=(r == NT - 1))
        nc.vector.tensor_copy(gram_sb[:, 0:128], psGS[:, 0:128])
        nc.vector.tensor_copy(gram_sb[:1, 128:256], psGS[:1, 128:256])

        # ---------- DRAM staging ----------
        p1_in, p1_out = [], []
        for c, (co, cw) in enumerate(CHUNKS):
            w = cw + (GEXT if c == 0 else 0)
            p1_in.append(dram.tile([128, w], bf16, tag=f"p1in{c}",
                                   name=f"p1in{c}"))
            p1_out.append(dram.tile([128, w], bf16, tag=f"p1out{c}",
                                    name=f"p1out{c}", addr_space=shared_space))

        # ================= pass1 / AR / pass2 =================
        henc = wtile([128, NLOC], f32, "henc")
        haT = wtile([128, NLOC], f32, "haT")
        sT = wtile([128, NLOC], f32, "sT")
        aTb = wtile([128, NLOC], bf16, "aTb")

        def emit_pass1(c):
            co, cw = CHUNKS[c]
            ut = u_tiles.pop(c)
            w = cw + (GEXT if c == 0 else 0)
            p1sb = p1sbp.tile([128, CHUNK + GEXT], bf16, tag="p1sb",
                              name=f"p1sb{c}")[:, :w]
            for q, (qo, qw) in enumerate(_splits(cw, 1024)):
                blocks = _splits(qw, 512)
                ps1 = [p1_tile(bw) for _, bw in blocks]
                for r, (ro, rw) in enumerate(ROWS):
                    for bi, (bo, bw) in enumerate(blocks):
                        nc.tensor.matmul(ps1[bi], h16[:rw, r, :],
                                         ut[(q, r)][:rw, bo:bo + bw],
                                         start=(r == 0), stop=(r == NT - 1))
                for bi, (bo, bw) in enumerate(blocks):
                    nc.vector.tensor_copy(p1sb[:, qo + bo:qo + bo + bw],
                                          ps1[bi])
            if c == 0:
                nc.vector.tensor_copy(p1sb[:, cw:cw + GEXT], gram_sb[:])
            nc.gpsimd.dma_start(out=p1_in[c][:], in_=p1sb)
            nc.gpsimd.collective_compute(
                "AllReduce", ALU.add, replica_groups=rg,
                ins=[p1_in[c].opt()], outs=[p1_out[c].opt()])

        uT_tiles = {}

        def emit_uT_loads(c):
            co, cw = CHUNKS[c]
            tl = []
            for t, (so, sw) in enumerate(_splits(cw, 128)):
                uTt = uTp.tile([128, NLOC], bf16, tag="uT",
                               name=f"uTl{c}_{t}")[:sw]
                nc.scalar.dma_start(out=uTt, in_=t_uT[co + so:co + so + sw, :])
                tl.append(uTt)
            uT_tiles[c] = tl

        def emit_pass2(c):
            co, cw = CHUNKS[c]
            subs = _splits(cw, 128)
            zc = zcp.tile([128, CHUNK + GEXT], bf16, tag="zc",
                          name=f"zc_{c}")[:, :cw]
            nc.sync.dma_start(out=zc, in_=p1_out[c][:, :cw])
            z16 = z16p.tile([128, (CHUNK + 127) // 128, D], bf16, tag="z16",
                            name=f"z16_{c}")
            for t, (so, sw) in enumerate(subs):
                psz = tb_tile(sw, 128)
                T(psz, zc[:, so:so + sw], identb[:])
                gidx = (co + so) // 128
                nc.vector.tensor_scalar(z16[:sw, t, :], psz,
                                        scalar1=ne[:sw, gidx:gidx + 1],
                                        scalar2=None, op0=ALU.mult)
            uTc = uT_tiles.pop(c)
            ps2 = [p2_tile(iw) for _, iw in BLK]
            for t, (so, sw) in enumerate(subs):
                for ib, (io, iw) in enumerate(BLK):
                    nc.tensor.matmul(ps2[ib], z16[:sw, t, :],
                                     uTc[t][:sw, io:io + iw],
                                     start=(t == 0), stop=(t == len(subs) - 1))
            for ib, (io, iw) in enumerate(BLK):
                if c == 0:
                    nc.vector.tensor_copy(henc[:, io:io + iw], ps2[ib])
                else:
                    nc.vector.tensor_add(henc[:, io:io + iw],
                                         henc[:, io:io + iw], ps2[ib])

        def emit_watt():
            co0, cw0 = CHUNKS[0]
            gkv = wtile([128, GEXT], bf16, "gkv")
            nc.gpsimd.dma_start(out=gkv[:], in_=p1_out[0][:, cw0:cw0 + GEXT])
            G_b = gkv[:, 0:128]
            s_row = gkv[:1, 128:256]
            psc = tb_tile(128, 1)
            T(psc, s_row, identb[:1, :1])
            s_col = rtile([128, 1], bf16, "s_col")
            nc.vector.tensor_copy(s_col[:], psc)
            # X1 = G Wv + s (.) bv   (shared by k1v and k2v)
            psX = mm_tile(128, 128)
            nc.tensor.matmul(psX, G_b, Wvb, start=True, stop=False)
            nc.tensor.matmul(psX, s_row, rowb[:1, 256:384], start=False, stop=True)
            X1b = wtile([128, 128], bf16, "X1b")
            nc.vector.tensor_copy(X1b[:], psX)
            # rrow = s^T Wv + N bv
            psr = mm_tile(1, 128)
            nc.tensor.matmul(psr, s_col[:], Wvb)
            rrow = rtile([1, 128], f32, "rrow")
            nc.vector.tensor_add(rrow[:], psr, roww[:1])
            rrow_b = rtile([1, 128], bf16, "rrow_b")
            nc.vector.tensor_copy(rrow_b[:], rrow[:])
            kvs = []
            for i, Wk in ((0, Wk1b), (1, Wk2b)):
                psK = mm_tile(128, 128)
                nc.tensor.matmul(psK, Wk, X1b[:], start=True, stop=False)
                nc.tensor.matmul(psK, rowb[:1, i * 128:(i + 1) * 128],
                                 rrow_b[:], start=False, stop=True)
                kv = wtile([128, 128], bf16, f"k{i+1}v_b")
                nc.vector.tensor_copy(kv[:], psK)
                kvs.append(kv)
            psW = mm_tile(128, 128)
            nc.tensor.matmul(psW, Wq1Tb, kvs[0][:], start=True, stop=False)
            nc.tensor.matmul(psW, Wq2Tsb, kvs[1][:], start=False, stop=True)
            Wattb = wtile([128, D], bf16, "Wattb")
            nc.vector.tensor_copy(Wattb[:], psW)
            psB = mm_tile(128, 1)
            nc.tensor.matmul(psB, kvs[0][:], colb[:, 0:1], start=True,
                             stop=False)
            nc.tensor.matmul(psB, kvs[1][:], colb[:, 1:2], start=False,
                             stop=True)
            batt_c = wtile([128, 1], f32, "batt_c")
            nc.vector.tensor_copy(batt_c[:], psB)
            return Wattb, batt_c

        # ---- pipeline schedule (see deadlock notes: uT0/uT1 triggers are
        # emitted early and fully admitted by the 32-buf pool; uT2..4 are
        # emitted between pass2 stages so their pool waits resolve via
        # pass2 consumption; gkv/zc reads stay off the scalar queue) ----
        emit_pass1(0)
        emit_u_loads(1)
        emit_uT_loads(0)
        emit_pass1(1)
        emit_u_loads(2)
        emit_pass1(2)
        emit_u_loads(3)
        Wattb, batt_c = emit_watt()
        # sT = Watt^T @ hnT + batt  (transposed layout)
        for bo, bw in BLK:
            pss = p2_tile(bw)
            nc.tensor.matmul(pss, Wattb[:], hnTb[:, bo:bo + bw])
            nc.vector.tensor_scalar(sT[:, bo:bo + bw], pss,
                                    scalar1=batt_c[:], scalar2=None,
                                    op0=ALU.add)
        lnT(sT, aTb, "s")
        emit_pass1(3)
        emit_u_loads(4)
        # haT = hT + Wo'^T @ aT + bo
        for bo, bw in BLK:
            psa = p2_tile(bw)
            nc.tensor.matmul(psa, Wob, aTb[:, bo:bo + bw])
            atmp = rowtmp.tile([128, 512], f32, tag="btmp", bufs=2,
                               name=f"atmp_{nc.next_id()}")[:, :bw]
            nc.vector.tensor_scalar(atmp, psa, scalar1=bo_c,
                                    scalar2=None, op0=ALU.add)
            nc.vector.tensor_add(haT[:, bo:bo + bw],
                                 hT[:, bo:bo + bw], atmp)
        emit_pass1(4)
        emit_uT_loads(1)
        emit_pass2(0)
        emit_uT_loads(2)
        emit_pass2(1)
        emit_uT_loads(3)
        emit_pass2(2)
        emit_uT_loads(4)
        emit_pass2(3)
        emit_pass2(4)

        # ================= epilogue =================
        nc.vector.tensor_add(haT[:], haT[:], henc[:])            # mhT
        fTb = wtile([128, NLOC], bf16, "fTb")
        lnT(haT, fTb, "f")
        gb = wtile([128, NLOC], bf16, "gb")
        for bo, bw in BLK:
            psg_ = p2_tile(bw)
            nc.tensor.matmul(psg_, W1pb, fTb[:, bo:bo + bw])
            nc.scalar.activation(gb[:, bo:bo + bw], psg_, AF.Gelu,
                                 bias=b1p_c[:])
        outT = sT  # sT is dead after lnT(sT); reuse its buffer
        for bo, bw in BLK:
            pso_ = p2_tile(bw)
            nc.tensor.matmul(pso_, f2wb, gb[:, bo:bo + bw])
            otmp = rowtmp.tile([128, 512], f32, tag="btmp", bufs=2,
                               name=f"otmp_{nc.next_id()}")[:, :bw]
            nc.vector.tensor_scalar(otmp, pso_, scalar1=f2b_c,
                                    scalar2=None, op0=ALU.add)
            nc.vector.tensor_add(outT[:, bo:bo + bw],
                                 haT[:, bo:bo + bw], otmp)
        nc.sync.dma_start(out=t_out[:], in_=outT[:])

    nc.compile()
    return nc


# ==================== host-side entry point ====================

_CACHED = {}


def _get_nc(N=N_FULL, NF=NF_FULL, CORES=CORES_FULL, CHUNK=CHUNK_FULL):
    key = (N, NF, CORES, CHUNK)
    if key not in _CACHED:
        _CACHED[key] = build_kernel(N, NF, CORES, CHUNK)
    return _CACHED[key]


def make_in_maps(inputs, N, CORES):
    import ml_dtypes

    NLOC = N // CORES
    NSUB = (N + 127) // 128
    bf = ml_dtypes.bfloat16
    f = {k: np.asarray(v, np.float64) for k, v in inputs.items()}
    LI = LAMBDA_INIT

    lam1 = np.exp(np.sum(f["lq1"] * f["lk1"]))
    lam2 = np.exp(np.sum(f["lq2"] * f["lk2"]))
    lam = lam1 - lam2 + LI
    mg, mb = f["mha_ln_g"], f["mha_ln_b"]
    Wk1 = f["k1_w"] * mg[:, None]; bk1 = mb @ f["k1_w"] + f["k1_b"]
    Wk2 = f["k2_w"] * mg[:, None]; bk2 = mb @ f["k2_w"] + f["k2_b"]
    Wv = f["v_w"] * mg[:, None]; bv = mb @ f["v_w"] + f["v_b"]
    Wq1 = f["q1_w"] * mg[:, None]; bq1 = mb @ f["q1_w"] + f["q1_b"]
    Wq2 = f["q2_w"] * mg[:, None]; bq2 = mb @ f["q2_w"] + f["q2_b"]
    Wob = f["attn_ln_g"][:, None] * f["out_w"] * (1 - LI)
    bo = (1 - LI) * (f["attn_ln_b"] @ f["out_w"]) + f["out_b"]
    W1p = f["ffn_ln_g"][:, None] * f["ffn1_w"]
    b1p = f["ffn_ln_b"] @ f["ffn1_w"] + f["ffn1_b"]

    kk = np.arange(1, 11)
    ang = f["e"][:, None] * kk / np.pi
    ne = (np.cos(ang) @ f["kan_a"] + np.sin(ang) @ f["kan_b"]
          + f["kan_bias"][0]) * f["alpha_w"][0, 0]
    ne_pad = np.zeros(NSUB * 128)
    ne_pad[:N] = ne
    ne_pm = np.ascontiguousarray(
        ne_pad.reshape(NSUB, 128).T.astype(np.float32))

    wb = np.concatenate(
        [f["fe_w1"].reshape(4, 128, 128)[i] for i in range(4)]
        + [f["fe_w2"], Wk1, Wk2, Wv, Wq1.T, -lam * Wq2.T, Wob, W1p,
           f["ffn2_w"]], axis=1)
    wb = np.ascontiguousarray(wb.astype(bf))
    colw = np.stack([f["fe_b1"], f["fe_b2"], bo, b1p, f["ffn2_b"],
                     np.zeros(128), np.zeros(128), np.zeros(128)], axis=1)
    colw = np.ascontiguousarray(colw.astype(np.float32))
    colb = np.ascontiguousarray(
        np.stack([bq1, -lam * bq2], axis=1).astype(bf))
    rowb = np.ascontiguousarray(
        np.concatenate([bk1, bk2, bv, np.zeros(128)])[None, :].astype(bf))
    roww = np.ascontiguousarray((N * bv)[None, :].astype(np.float32))

    x = np.asarray(inputs["x"], np.float32)
    u = np.asarray(inputs["u"], np.float32)
    in_maps = []
    for c in range(CORES):
        sh = u[c * NLOC:(c + 1) * NLOC]
        m = {
            "u": np.ascontiguousarray(sh.astype(bf)),
            "uT": np.ascontiguousarray(sh.T.astype(bf)),
            "xT": np.ascontiguousarray(
                x[c * NLOC:(c + 1) * NLOC].T.astype(bf)),
            "wb": wb, "colw": colw, "colb": colb, "rowb": rowb,
            "roww": roww, "ne": ne_pm,
        }
        in_maps.append(m)
    return in_maps


def assemble_out(res, CORES=CORES_FULL):
    # per-core outputs are [D, NLOC] (transposed); transpose + concat rows
    return np.concatenate(
        [np.asarray(res.results[c]["out"]).T for c in range(CORES)],
        axis=0).astype(np.float32)


def kernel(**inputs):
    from concourse import bass_utils

    nc = _get_nc()
    in_maps = make_in_maps(inputs, N_FULL, CORES_FULL)
    res = bass_utils.run_bass_kernel_spmd(nc, in_maps,
                                          core_ids=list(range(CORES_FULL)))
    return assemble_out(res)


if __name__ == "__main__":
    build_kernel()
    print("build ok")


# revision 16
# speedup vs baseline: 1.2451x; 1.0518x over previous
"""Trainium2 Bass kernel for nn_NoFoDifformer_FourierKAN (8-core SPMD), v4.

Sharding: u and nodes row-wise across 8 cores (1250 rows each). The z = u^T h
partial sums are all-reduced per column-chunk (bf16); the [d,d] Gram matrix of
the normalized activations rides inside the first chunk's all-reduce. Small
weights are pre-folded and packed on the HOST (LayerNorm affines folded into
projections, lambda and the FourierKAN filter evaluated in numpy) so the
device preamble is a handful of panel DMAs. Per-core outputs are [d, n_loc]
(transposed) and transposed+concatenated on the host.

v4 scheduling notes (engine FIFOs are in-order; a blocked DMA trigger blocks
everything behind it on that queue, so queue assignment is load-bearing):
- sync queue: xT, all u quarter-tiles, uT0, gkv (waits AR0), uT1..4, out.
- scalar queue: activations + zc readbacks only (no uT triggers -> the LN
  Sqrt chain can never deadlock against uT pool recycling).
- gpsimd queue: weight panels, p1 staging writes, AR triggers (nothing that
  waits on an AR result, so the AR chain stays dense).
- attention epilogue (watt/sT/haT) is emitted after pass1(4) so its LN never
  sits between pass1 stages in the Tensor FIFO.
- descending chunk plan [3072,3072,2048,1024,784]: big early chunks overlap
  the startup barrier; small late ARs shrink the tail.
"""

import numpy as np

N_FULL = 10000
NF_FULL = 512
D = 128
CORES_FULL = 8
LAMBDA_INIT = 0.2
GEXT = 256  # extra AR columns on chunk 0 for (G | s)
CH_PLAN = [3072, 3072, 2048, 1024, 784]


def _splits(total, step):
    return [(o, min(step, total - o)) for o in range(0, total, step)]


def build_kernel(N=N_FULL, NF=NF_FULL, CORES=CORES_FULL):
    import concourse.bacc as bacc
    import concourse.tile as tile
    from concourse import mybir
    from concourse.masks import make_identity
    from contextlib import ExitStack

    dt = mybir.dt
    f32 = dt.float32
    bf16 = dt.bfloat16
    AF = mybir.ActivationFunctionType
    ALU = mybir.AluOpType

    NLOC = N // CORES                   # 1250
    ROWS = _splits(NLOC, 128)           # 9x128 + 98
    NT = len(ROWS)
    KX = NF // 128
    assert sum(CH_PLAN) == N
    CHUNKS = []
    off = 0
    for w in CH_PLAN:
        CHUNKS.append((off, w))
        off += w
    NCH = len(CHUNKS)
    CHMAX = max(CH_PLAN)
    NSUB = (N + 127) // 128             # 79
    BLK = _splits(NLOC, 512)            # [d, NLOC] op blocks
    rg = [list(range(CORES))]
    shared_space = "Shared" if CORES > 4 else "Local"

    nc = bacc.Bacc("TRN2", target_bir_lowering=False, debug=False,
                   num_devices=CORES)

    # ---------------- DRAM I/O ----------------
    t_u = nc.dram_tensor("u", [NLOC, N], bf16, kind="ExternalInput")
    t_uT = nc.dram_tensor("uT", [N, NLOC], bf16, kind="ExternalInput")
    t_xT = nc.dram_tensor("xT", [NF, NLOC], bf16, kind="ExternalInput")
    t_wb = nc.dram_tensor("wb", [128, 13 * 128], bf16, kind="ExternalInput")
    t_colw = nc.dram_tensor("colw", [128, 8], f32, kind="ExternalInput")
    t_colb = nc.dram_tensor("colb", [128, 2], bf16, kind="ExternalInput")
    t_rowb = nc.dram_tensor("rowb", [1, 4 * 128], bf16, kind="ExternalInput")
    t_roww = nc.dram_tensor("roww", [1, 128], f32, kind="ExternalInput")
    t_ne = nc.dram_tensor("ne", [128, NSUB], f32, kind="ExternalInput")
    t_out = nc.dram_tensor("out", [D, NLOC], f32, kind="ExternalOutput")

    with tile.TileContext(nc) as tc, ExitStack() as ctx:
        wpool = ctx.enter_context(tc.tile_pool(name="wpool", bufs=1))
        rowtmp = ctx.enter_context(tc.tile_pool(name="rowtmp", bufs=3))
        ustream = ctx.enter_context(tc.tile_pool(name="ustream", bufs=12))
        uTp = ctx.enter_context(tc.tile_pool(name="uTp", bufs=20))
        zcp = ctx.enter_context(tc.tile_pool(name="zcp", bufs=2))
        z16p = ctx.enter_context(tc.tile_pool(name="z16p", bufs=2))
        p1sbp = ctx.enter_context(tc.tile_pool(name="p1sbp", bufs=2))
        dram = ctx.enter_context(tc.tile_pool(name="dram", bufs=1, space="DRAM"))
        ps_p1 = ctx.enter_context(tc.tile_pool(name="ps_p1", bufs=2, space="PSUM"))
        ps_p2 = ctx.enter_context(tc.tile_pool(name="ps_p2", bufs=3, space="PSUM"))
        ps_mm = ctx.enter_context(tc.tile_pool(name="ps_mm", bufs=1, space="PSUM"))
        ps_t = ctx.enter_context(tc.tile_pool(name="ps_t", bufs=2, space="PSUM"))

        def p1_tile(w):
            return ps_p1.tile([128, 512], f32, tag="p1",
                              name=f"p1_{nc.next_id()}")[:, :w]

        def p2_tile(w):
            return ps_p2.tile([128, 512], f32, tag="p2",
                              name=f"p2_{nc.next_id()}")[:, :w]

        def mm_tile(p, w):
            return ps_mm.tile([128, 512], f32, tag="mmp",
                              name=f"mm_{nc.next_id()}")[:p, :w]

        def tb_tile(p, w):
            return ps_t.tile([128, 128], bf16, tag="pstb",
                             name=f"pstb_{nc.next_id()}")[:p, :w]

        def wtile(shape, dtype, name):
            return wpool.tile(shape, dtype, tag=name, name=name)

        def rtile(shape, dtype, tag):
            return rowtmp.tile(shape, dtype, tag=tag,
                               name=f"{tag}_{nc.next_id()}")

        def T(out_psum, in_sbuf, identity):
            nc.tensor.matmul(out_psum, in_sbuf, identity, is_transpose=True)

        # ================= constants & weights =================
        identb = wtile([128, 128], bf16, "identb")
        make_identity(nc, identb[:])
        ones_row_b = wtile([1, 128], bf16, "ones_row_b")
        nc.vector.memset(ones_row_b[:], 1.0)
        ones_col_b = wtile([128, 1], bf16, "ones_col_b")
        nc.vector.memset(ones_col_b[:], 1.0)
        oinv_col_b = wtile([128, 1], bf16, "oinv_col_b")
        nc.vector.memset(oinv_col_b[:], 1.0 / 128.0)
        eps_col = wtile([128, 1], f32, "eps_col")
        nc.vector.memset(eps_col[:], 1e-5)

        wb = wtile([128, 13 * 128], bf16, "wb")
        nc.gpsimd.dma_start(out=wb[:], in_=t_wb[:])
        colw = wtile([128, 8], f32, "colw")
        nc.gpsimd.dma_start(out=colw[:], in_=t_colw[:])
        colb = wtile([128, 2], bf16, "colb")
        nc.gpsimd.dma_start(out=colb[:], in_=t_colb[:])
        rowb = wtile([1, 4 * 128], bf16, "rowb")
        nc.gpsimd.dma_start(out=rowb[:], in_=t_rowb[:])
        roww = wtile([1, 128], f32, "roww")
        nc.gpsimd.dma_start(out=roww[:], in_=t_roww[:])
        ne = wtile([128, NSUB], f32, "ne")
        nc.gpsimd.dma_start(out=ne[:], in_=t_ne[:])

        def P(i):  # weight panel i of wb
            return wb[:, i * 128:(i + 1) * 128]
        few2b = P(4)
        Wk1b, Wk2b, Wvb = P(5), P(6), P(7)
        Wq1Tb, Wq2Tsb = P(8), P(9)
        Wob, W1pb, f2wb = P(10), P(11), P(12)
        feb1_c = colw[:, 0:1]
        feb2_c = colw[:, 1:2]
        bo_c = colw[:, 2:3]
        b1p_c = colw[:, 3:4]
        f2b_c = colw[:, 4:5]

        # xT into SBUF (4 partition k-tiles)
        xT4 = wtile([128, KX, NLOC], bf16, "xT4")
        for kt in range(KX):
            nc.sync.dma_start(out=xT4[:, kt, :],
                              in_=t_xT[kt * 128:(kt + 1) * 128, :])

        # ---------- u streaming loads (quarter tiles, sync queue) ----------
        u_tiles = {}

        def emit_u_loads(c):
            co, cw = CHUNKS[c]
            tiles = {}
            for q, (qo, qw) in enumerate(_splits(cw, 1024)):
                for r, (ro, rw) in enumerate(ROWS):
                    ut = ustream.tile([128, 1024], bf16, tag="u",
                                      name=f"u{c}_{q}_{r}")[:rw, :qw]
                    nc.sync.dma_start(
                        out=ut, in_=t_u[ro:ro + rw, co + qo:co + qo + qw])
                    tiles[(q, r)] = ut
            u_tiles[c] = tiles

        uT_tiles = {}

        def emit_uT_loads(c):
            co, cw = CHUNKS[c]
            tl = []
            for t, (so, sw) in enumerate(_splits(cw, 128)):
                uTt = uTp.tile([128, NLOC], bf16, tag="uT",
                               name=f"uTl{c}_{t}")[:sw]
                nc.sync.dma_start(out=uTt, in_=t_uT[co + so:co + so + sw, :])
                tl.append(uTt)
            uT_tiles[c] = tl

        emit_u_loads(0)
        emit_u_loads(1)
        emit_uT_loads(0)
        emit_u_loads(2)
        emit_u_loads(3)
        emit_u_loads(4)

        # ================= phase A: feature encoder (transposed) ==========
        hT = wtile([128, NLOC], f32, "hT")
        hTb = wtile([128, NLOC], bf16, "hTb")
        h16 = wtile([128, NT, D], bf16, "h16")
        for go, gw in BLK:
            psh1 = p2_tile(gw)
            for kt in range(KX):
                nc.tensor.matmul(psh1, P(kt), xT4[:, kt, go:go + gw],
                                 start=(kt == 0), stop=(kt == KX - 1))
            h1t = rtile([128, 512], bf16, "h1t")[:, :gw]
            nc.scalar.activation(h1t, psh1, AF.Relu, bias=feb1_c[:])
            pshT = p2_tile(gw)
            nc.tensor.matmul(pshT, few2b, h1t)
            nc.vector.tensor_scalar(hT[:, go:go + gw], pshT,
                                    scalar1=feb2_c, scalar2=None, op0=ALU.add)
            nc.scalar.activation(hTb[:, go:go + gw], hT[:, go:go + gw],
                                 AF.Copy)
        for r, (ro, rw) in enumerate(ROWS):
            pst = tb_tile(rw, 128)
            T(pst, hTb[:, ro:ro + rw], identb[:])
            nc.vector.tensor_copy(h16[:rw, r, :], pst)

        # ---------- transposed-layout LayerNorm helper ----------
        def lnT(x_sb, out_bf, pfx, xb=None):
            if xb is None:
                xb = wpool.tile([128, NLOC], bf16, tag="ln_xb",
                                name=f"{pfx}_xb")
                nc.scalar.activation(xb[:], x_sb[:], AF.Copy)
            x2b = wpool.tile([128, NLOC], bf16, tag="ln_x2b",
                             name=f"{pfx}_x2b")
            nc.vector.tensor_mul(x2b[:], x_sb[:], x_sb[:])

            def frow(tag, dt_):
                return rowtmp.tile([1, NLOC], dt_, tag=tag, bufs=2,
                                   name=f"{tag}_{nc.next_id()}")
            mrow = frow("ln_m", f32)
            rsrow = frow("ln_r", f32)
            for bo, bw in BLK:
                psm = mm_tile(1, bw)
                nc.tensor.matmul(psm, oinv_col_b[:], xb[:, bo:bo + bw])
                nc.vector.tensor_copy(mrow[:, bo:bo + bw], psm)
                psq_ = mm_tile(1, bw)
                nc.tensor.matmul(psq_, oinv_col_b[:], x2b[:, bo:bo + bw])
                nc.vector.tensor_mul(rsrow[:, bo:bo + bw],
                                     mrow[:, bo:bo + bw], mrow[:, bo:bo + bw])
                nc.vector.tensor_sub(rsrow[:, bo:bo + bw], psq_,
                                     rsrow[:, bo:bo + bw])         # var
            nc.scalar.activation(rsrow[:], rsrow[:], AF.Sqrt,
                                 bias=eps_col[:1])
            nc.vector.reciprocal(rsrow[:], rsrow[:])               # 1/sqrt
            m_b = frow("ln_mb", bf16)
            nc.vector.tensor_copy(m_b[:], mrow[:])
            rs_b = frow("ln_rb", bf16)
            nc.vector.tensor_copy(rs_b[:], rsrow[:])
            for bo, bw in BLK:
                psM = p2_tile(bw)
                nc.tensor.matmul(psM, ones_row_b[:], m_b[:, bo:bo + bw])
                psR = p2_tile(bw)
                nc.tensor.matmul(psR, ones_row_b[:], rs_b[:, bo:bo + bw])
                dtmp = rowtmp.tile([128, 512], f32, tag="btmp", bufs=2,
                                   name=f"lnd_{nc.next_id()}")[:, :bw]
                nc.vector.tensor_sub(dtmp, x_sb[:, bo:bo + bw], psM)
                nc.vector.tensor_mul(out_bf[:, bo:bo + bw], dtmp, psR)

        hnTb = wtile([128, NLOC], bf16, "hnTb")
        lnT(hT, hnTb, "hn", xb=hTb)

        # ---------- Gram: G = hn^T hn, s = hn^T 1 (into gram_sb bf16) ------
        gram_sb = wtile([128, GEXT], bf16, "gram_sb")
        psGS = ps_mm.tile([128, 512], f32, tag="mmp", name="psGS")
        for r, (ro, rw) in enumerate(ROWS):
            pst = tb_tile(rw, 128)
            T(pst, hnTb[:, ro:ro + rw], identb[:])
            hn_r = rtile([128, 128], bf16, "hn_r")[:rw]
            nc.vector.tensor_copy(hn_r, pst)
            nc.tensor.matmul(psGS[:, 0:128], hn_r, hn_r,
                             start=(r == 0), stop=(r == NT - 1))
            nc.tensor.matmul(psGS[:1, 128:256], ones_col_b[:rw], hn_r,
                             start=(r == 0), stop=(r == NT - 1))
        nc.vector.tensor_copy(gram_sb[:, 0:128], psGS[:, 0:128])
        nc.vector.tensor_copy(gram_sb[:1, 128:256], psGS[:1, 128:256])

        # ---------- DRAM staging ----------
        p1_in, p1_out = [], []
        for c, (co, cw) in enumerate(CHUNKS):
            w = cw + (GEXT if c == 0 else 0)
            p1_in.append(dram.tile([128, w], bf16, tag=f"p1in{c}",
                                   name=f"p1in{c}"))
            p1_out.append(dram.tile([128, w], bf16, tag=f"p1out{c}",
                                    name=f"p1out{c}", addr_space=shared_space))

        haT = wtile([128, NLOC], f32, "haT")
        sT = wtile([128, NLOC], f32, "sT")
        sTb = wtile([128, NLOC], bf16, "sTb")
        aTb = wtile([128, NLOC], bf16, "aTb")

        def emit_pass1(c):
            co, cw = CHUNKS[c]
            ut = u_tiles.pop(c)
            w = cw + (GEXT if c == 0 else 0)
            p1sb = p1sbp.tile([128, CHMAX + GEXT], bf16, tag="p1sb",
                              name=f"p1sb{c}")[:, :w]
            for q, (qo, qw) in enumerate(_splits(cw, 1024)):
                blocks = _splits(qw, 512)
                ps1 = [p1_tile(bw) for _, bw in blocks]
                for r, (ro, rw) in enumerate(ROWS):
                    for bi, (bo, bw) in enumerate(blocks):
                        nc.tensor.matmul(ps1[bi], h16[:rw, r, :],
                                         ut[(q, r)][:rw, bo:bo + bw],
                                         start=(r == 0), stop=(r == NT - 1))
                for bi, (bo, bw) in enumerate(blocks):
                    nc.vector.tensor_copy(p1sb[:, qo + bo:qo + bo + bw],
                                          ps1[bi])
            if c == 0:
                nc.vector.tensor_copy(p1sb[:, cw:cw + GEXT], gram_sb[:])
            nc.gpsimd.dma_start(out=p1_in[c][:], in_=p1sb)
            nc.gpsimd.collective_compute(
                "AllReduce", ALU.add, replica_groups=rg,
                ins=[p1_in[c].opt()], outs=[p1_out[c].opt()])

        def emit_pass2(c):
            co, cw = CHUNKS[c]
            subs = _splits(cw, 128)
            zc = zcp.tile([128, CHMAX + GEXT], bf16, tag="zc",
                          name=f"zc_{c}")[:, :cw]
            nc.scalar.dma_start(out=zc, in_=p1_out[c][:, :cw])
            z16 = z16p.tile([128, (CHMAX + 127) // 128, D], bf16, tag="z16",
                            name=f"z16_{c}")
            for t, (so, sw) in enumerate(subs):
                psz = tb_tile(sw, 128)
                T(psz, zc[:, so:so + sw], identb[:])
                gidx = (co + so) // 128
                nc.vector.tensor_scalar(z16[:sw, t, :], psz,
                                        scalar1=ne[:sw, gidx:gidx + 1],
                                        scalar2=None, op0=ALU.mult)
            uTc = uT_tiles.pop(c)
            ps2 = [p2_tile(iw) for _, iw in BLK]
            for t, (so, sw) in enumerate(subs):
                for ib, (io, iw) in enumerate(BLK):
                    nc.tensor.matmul(ps2[ib], z16[:sw, t, :],
                                     uTc[t][:sw, io:io + iw],
                                     start=(t == 0), stop=(t == len(subs) - 1))
            for ib, (io, iw) in enumerate(BLK):
                nc.vector.tensor_add(haT[:, io:io + iw],
                                     haT[:, io:io + iw], ps2[ib])

        def emit_att():
            # post-AR0 attention path: gram -> k1v/k2v -> Watt -> sT -> aT -> haT
            co0, cw0 = CHUNKS[0]
            gkv = wtile([128, GEXT], bf16, "gkv")
            nc.sync.dma_start(out=gkv[:], in_=p1_out[0][:, cw0:cw0 + GEXT])
            G_b = gkv[:, 0:128]
            s_row = gkv[:1, 128:256]
            psc = tb_tile(128, 1)
            T(psc, s_row, identb[:1, :1])
            s_col = rtile([128, 1], bf16, "s_col")
            nc.vector.tensor_copy(s_col[:], psc)
            # X1 = G Wv + s (.) bv   (shared by k1v and k2v)
            psX = mm_tile(128, 128)
            nc.tensor.matmul(psX, G_b, Wvb, start=True, stop=False)
            nc.tensor.matmul(psX, s_row, rowb[:1, 256:384], start=False,
                             stop=True)
            X1b = wtile([128, 128], bf16, "X1b")
            nc.vector.tensor_copy(X1b[:], psX)
            # rrow = s^T Wv + N bv
            psr = mm_tile(1, 128)
            nc.tensor.matmul(psr, s_col[:], Wvb)
            rrow = rtile([1, 128], f32, "rrow")
            nc.vector.tensor_add(rrow[:], psr, roww[:1])
            rrow_b = rtile([1, 128], bf16, "rrow_b")
            nc.vector.tensor_copy(rrow_b[:], rrow[:])
            kvs = []
            for i, Wk in ((0, Wk1b), (1, Wk2b)):
                psK = mm_tile(128, 128)
                nc.tensor.matmul(psK, Wk, X1b[:], start=True, stop=False)
                nc.tensor.matmul(psK, rowb[:1, i * 128:(i + 1) * 128],
                                 rrow_b[:], start=False, stop=True)
                kv = wtile([128, 128], bf16, f"k{i+1}v_b")
                nc.vector.tensor_copy(kv[:], psK)
                kvs.append(kv)
            psW = mm_tile(128, 128)
            nc.tensor.matmul(psW, Wq1Tb, kvs[0][:], start=True, stop=False)
            nc.tensor.matmul(psW, Wq2Tsb, kvs[1][:], start=False, stop=True)
            Wattb = wtile([128, D], bf16, "Wattb")
            nc.vector.tensor_copy(Wattb[:], psW)
            psB = mm_tile(128, 1)
            nc.tensor.matmul(psB, kvs[0][:], colb[:, 0:1], start=True,
                             stop=False)
            nc.tensor.matmul(psB, kvs[1][:], colb[:, 1:2], start=False,
                             stop=True)
            batt_c = wtile([128, 1], f32, "batt_c")
            nc.vector.tensor_copy(batt_c[:], psB)
            # sT = Watt^T @ hnT + batt  (transposed layout)
            for bo, bw in BLK:
                pss = p2_tile(bw)
                nc.tensor.matmul(pss, Wattb[:], hnTb[:, bo:bo + bw])
                nc.vector.tensor_scalar(sT[:, bo:bo + bw], pss,
                                        scalar1=batt_c[:], scalar2=None,
                                        op0=ALU.add)
                nc.vector.tensor_copy(sTb[:, bo:bo + bw], sT[:, bo:bo + bw])
            lnT(sT, aTb, "s", xb=sTb)
            # haT = hT + Wo'^T @ aT + bo
            for bo, bw in BLK:
                psa = p2_tile(bw)
                nc.tensor.matmul(psa, Wob, aTb[:, bo:bo + bw])
                atmp = rowtmp.tile([128, 512], f32, tag="btmp", bufs=2,
                                   name=f"atmp_{nc.next_id()}")[:, :bw]
                nc.vector.tensor_scalar(atmp, psa, scalar1=bo_c,
                                        scalar2=None, op0=ALU.add)
                nc.vector.tensor_add(haT[:, bo:bo + bw],
                                     hT[:, bo:bo + bw], atmp)

        # ---- pipeline ----
        emit_pass1(0)
        emit_pass1(1)
        emit_pass1(2)
        emit_pass1(3)
        emit_pass1(4)
        # gkv + uT1 go on sync AFTER all u triggers (audit in header)
        emit_att()
        emit_uT_loads(1)
        emit_pass2(0)
        emit_uT_loads(2)
        emit_pass2(1)
        emit_uT_loads(3)
        emit_pass2(2)
        emit_uT_loads(4)
        emit_pass2(3)
        emit_pass2(4)

        # ================= final epilogue (per-block, early out DMA) ======
        fTb = wtile([128, NLOC], bf16, "fTb")
        lnT(haT, fTb, "f")
        outT = sT  # sT is dead after lnT(sT); reuse its buffer
        for bo, bw in BLK:
            psg_ = p2_tile(bw)
            nc.tensor.matmul(psg_, W1pb, fTb[:, bo:bo + bw])
            gb_ = rtile([128, 512], bf16, "gb")[:, :bw]
            nc.scalar.activation(gb_, psg_, AF.Gelu, bias=b1p_c[:])
            pso_ = p2_tile(bw)
            nc.tensor.matmul(pso_, f2wb, gb_)
            otmp = rowtmp.tile([128, 512], f32, tag="btmp", bufs=2,
                               name=f"otmp_{nc.next_id()}")[:, :bw]
            nc.vector.tensor_scalar(otmp, pso_, scalar1=f2b_c,
                                    scalar2=None, op0=ALU.add)
            nc.vector.tensor_add(outT[:, bo:bo + bw],
                                 haT[:, bo:bo + bw], otmp)
            nc.sync.dma_start(out=t_out[:, bo:bo + bw],
                              in_=outT[:, bo:bo + bw])

    nc.compile()
    return nc


# ==================== host-side entry point ====================

_CACHED = {}


def _get_nc(N=N_FULL, NF=NF_FULL, CORES=CORES_FULL):
    key = (N, NF, CORES)
    if key not in _CACHED:
        _CACHED[key] = build_kernel(N, NF, CORES)
    return _CACHED[key]


def make_in_maps(inputs, N, CORES):
    import ml_dtypes

    NLOC = N // CORES
    NSUB = (N + 127) // 128
    bf = ml_dtypes.bfloat16
    f = {k: np.asarray(v, np.float64) for k, v in inputs.items()}
    LI = LAMBDA_INIT

    lam1 = np.exp(np.sum(f["lq1"] * f["lk1"]))
    lam2 = np.exp(np.sum(f["lq2"] * f["lk2"]))
    lam = lam1 - lam2 + LI
    mg, mb = f["mha_ln_g"], f["mha_ln_b"]
    Wk1 = f["k1_w"] * mg[:, None]; bk1 = mb @ f["k1_w"] + f["k1_b"]
    Wk2 = f["k2_w"] * mg[:, None]; bk2 = mb @ f["k2_w"] + f["k2_b"]
    Wv = f["v_w"] * mg[:, None]; bv = mb @ f["v_w"] + f["v_b"]
    Wq1 = f["q1_w"] * mg[:, None]; bq1 = mb @ f["q1_w"] + f["q1_b"]
    Wq2 = f["q2_w"] * mg[:, None]; bq2 = mb @ f["q2_w"] + f["q2_b"]
    Wob = f["attn_ln_g"][:, None] * f["out_w"] * (1 - LI)
    bo = (1 - LI) * (f["attn_ln_b"] @ f["out_w"]) + f["out_b"]
    W1p = f["ffn_ln_g"][:, None] * f["ffn1_w"]
    b1p = f["ffn_ln_b"] @ f["ffn1_w"] + f["ffn1_b"]

    kk = np.arange(1, 11)
    ang = f["e"][:, None] * kk / np.pi
    ne = (np.cos(ang) @ f["kan_a"] + np.sin(ang) @ f["kan_b"]
          + f["kan_bias"][0]) * f["alpha_w"][0, 0]
    ne_pad = np.zeros(NSUB * 128)
    ne_pad[:N] = ne
    ne_pm = np.ascontiguousarray(
        ne_pad.reshape(NSUB, 128).T.astype(np.float32))

    wb = np.concatenate(
        [f["fe_w1"].reshape(4, 128, 128)[i] for i in range(4)]
        + [f["fe_w2"], Wk1, Wk2, Wv, Wq1.T, -lam * Wq2.T, Wob, W1p,
           f["ffn2_w"]], axis=1)
    wb = np.ascontiguousarray(wb.astype(bf))
    colw = np.stack([f["fe_b1"], f["fe_b2"], bo, b1p, f["ffn2_b"],
                     np.zeros(128), np.zeros(128), np.zeros(128)], axis=1)
    colw = np.ascontiguousarray(colw.astype(np.float32))
    colb = np.ascontiguousarray(
        np.stack([bq1, -lam * bq2], axis=1).astype(bf))
    rowb = np.ascontiguousarray(
        np.concatenate([bk1, bk2, bv, np.zeros(128)])[None, :].astype(bf))
    roww = np.ascontiguousarray((N * bv)[None, :].astype(np.float32))

    x = np.asarray(inputs["x"], np.float32)
    u = np.asarray(inputs["u"], np.float32)
    in_maps = []
    for c in range(CORES):
        sh = u[c * NLOC:(c + 1) * NLOC]
        m = {
            "u": np.ascontiguousarray(sh.astype(bf)),
            "uT": np.ascontiguousarray(sh.T.astype(bf)),
            "xT": np.ascontiguousarray(
                x[c * NLOC:(c + 1) * NLOC].T.astype(bf)),
            "wb": wb, "colw": colw, "colb": colb, "rowb": rowb,
            "roww": roww, "ne": ne_pm,
        }
        in_maps.append(m)
    return in_maps


def assemble_out(res, CORES=CORES_FULL):
    # per-core outputs are [D, NLOC] (transposed); transpose + concat rows
    return np.concatenate(
        [np.asarray(res.results[c]["out"]).T for c in range(CORES)],
        axis=0).astype(np.float32)


def kernel(**inputs):
    from concourse import bass_utils

    nc = _get_nc()
    in_maps = make_in_maps(inputs, N_FULL, CORES_FULL)
    res = bass_utils.run_bass_kernel_spmd(nc, in_maps,
                                          core_ids=list(range(CORES_FULL)))
    return assemble_out(res)


if __name__ == "__main__":
    build_kernel()
    print("build ok")
